# revision 3
# baseline (speedup 1.0000x reference)
"""Trainium2 Bass kernel for nn_AttentionLayer (B=2, S=4096, D=512, H=8, E=64).

Strategy: head-parallel over 8 NeuronCores. Each core computes one head's
Q/K/V projections + attention + output projection for the full sequence,
then a ReduceScatter (per batch) sums the per-head partial outputs and
shards the summed rows across cores; each core runs the (replicated) MLP
on its row-slice and writes its slice of the final output.

Layout notes (everything is "feature on partitions" until the MLP output):
  x^T tiles  [d(128), s]   via DMA-transpose of host-pre-cast bf16 x
  Q^T / K^T  [e(64),  s]   packed in one [128, S] tile (rows 0:64 = Q^T,
                           64:128 = K^T)
  V1 tiles   [k(128), 65]  V with a ones-column appended so the attention
                           AV matmul also produces the softmax denominator
  scores^T   [k(128), q]   exp() applied on PSUM eviction (no max-subtract:
                           inputs are bounded so exp cannot overflow)
  y^T        [d(128), q]   normalized after the wo projection (divide by
                           denominator commutes with the linear projection)

Exact algebra folds (no approximation): bk drops out of softmax entirely;
bv and bo are folded into an effective MLP b1 on the host (b1_eff), since
sum_k softmax = 1 and the MLP consumes y linearly before relu.
"""

import numpy as np
import ml_dtypes

import concourse.bass as bass
import concourse.bacc as bacc
import concourse.mybir as mybir
import concourse.tile as tile

F32 = mybir.dt.float32
BF16 = mybir.dt.bfloat16
AF = mybir.ActivationFunctionType

B, S, D, H, E = 2, 4096, 512, 8, 64
N_CORES = 8


def build(B_=B, S_=S, debug=False):
    """Build the per-core SPMD graph. All 8 cores run the identical graph;
    per-head behavior comes from the per-core weight shards."""
    QC = S_ // N_CORES          # per-core slice rows per batch == q-chunk
    KT = S_ // 128              # number of 128-row key tiles
    DJ = D // 128               # 4 chunks of the hidden dim
    NSC = S_ // 512             # phase-A s-chunks of 512 rows
    assert QC % 128 == 0 and QC <= 512 and S_ % 512 == 0

    nc = bacc.Bacc(None, target_bir_lowering=False, debug=debug,
                   num_devices=N_CORES)

    xb = nc.declare_dram_parameter("xb", [B_ * S_, D], BF16, False)
    wq = nc.declare_dram_parameter("wq", [D, E], BF16, False)
    wk = nc.declare_dram_parameter("wk", [D, E], BF16, False)
    wv = nc.declare_dram_parameter("wv", [D, E], BF16, False)
    bq = nc.declare_dram_parameter("bq", [E, 1], F32, False)
    wo = nc.declare_dram_parameter("wo", [E, D], BF16, False)
    w1 = nc.declare_dram_parameter("w1", [D, D], BF16, False)
    b1e = nc.declare_dram_parameter("b1e", [D, 1], F32, False)
    w2 = nc.declare_dram_parameter("w2", [D, D], BF16, False)
    b2 = nc.declare_dram_parameter("b2", [1, D], F32, False)
    out = nc.declare_dram_parameter("out", [B_ * QC, D], F32, True)

    # Internal DRAM bounce buffers for the collectives.
    y_bounce = [nc.dram_tensor(f"y_bounce{b}", [N_CORES, D, QC], F32)
                for b in range(B_)]
    rs_out = [nc.dram_tensor(f"rs_out{b}", [D, QC], F32)
              for b in range(B_)]
    groups = [list(range(N_CORES))]

    with tile.TileContext(nc) as tc:
        from contextlib import ExitStack
        with ExitStack() as ctx:
            consts = ctx.enter_context(tc.tile_pool(name="consts", bufs=1))
            xT_pool = ctx.enter_context(tc.tile_pool(name="xT", bufs=2))
            qk_pool = ctx.enter_context(tc.tile_pool(name="qk", bufs=2))
            v1_pool = ctx.enter_context(tc.tile_pool(name="v1", bufs=2))
            exp_pool = ctx.enter_context(tc.tile_pool(name="exp", bufs=3))
            ctx_pool = ctx.enter_context(tc.tile_pool(name="ctxp", bufs=2))
            y_pool = ctx.enter_context(tc.tile_pool(name="y", bufs=4))
            mlp_pool = ctx.enter_context(tc.tile_pool(name="mlp", bufs=2))
            misc_pool = ctx.enter_context(tc.tile_pool(name="misc", bufs=2))
            ps_a = ctx.enter_context(
                tc.tile_pool(name="ps_a", bufs=2, space="PSUM"))
            ps_s = ctx.enter_context(
                tc.tile_pool(name="ps_s", bufs=2, space="PSUM"))
            ps_c = ctx.enter_context(
                tc.tile_pool(name="ps_c", bufs=2, space="PSUM"))

            # ---- constants / weights into SBUF ----
            wq_sb = consts.tile([128, DJ, E], BF16, tag="wq")
            wk_sb = consts.tile([128, DJ, E], BF16, tag="wk")
            wv_sb = consts.tile([128, DJ, E], BF16, tag="wv")
            nc.sync.dma_start(wq_sb[:], wq.rearrange("(a p) e -> p a e", p=128))
            nc.sync.dma_start(wk_sb[:], wk.rearrange("(a p) e -> p a e", p=128))
            nc.sync.dma_start(wv_sb[:], wv.rearrange("(a p) e -> p a e", p=128))
            bq_sb = consts.tile([E, 1], F32, tag="bq")
            nc.sync.dma_start(bq_sb[:], bq[:, :])
            wo_sb = consts.tile([E, D], BF16, tag="wo")
            nc.sync.dma_start(wo_sb[:], wo[:, :])
            w1_sb = consts.tile([128, DJ, D], BF16, tag="w1")
            nc.sync.dma_start(w1_sb[:], w1.rearrange("(a p) m -> p a m", p=128))
            w2_sb = consts.tile([128, DJ, D], BF16, tag="w2")
            nc.sync.dma_start(w2_sb[:], w2.rearrange("(a p) m -> p a m", p=128))
            b1e_sb = consts.tile([128, DJ], F32, tag="b1e")
            nc.sync.dma_start(b1e_sb[:], b1e.rearrange("(a p) o -> p (a o)", p=128))
            b2row_sb = consts.tile([1, D], F32, tag="b2row")
            nc.sync.dma_start(b2row_sb[:], b2[:, :])
            b2b_sb = consts.tile([128, D], F32, tag="b2b")
            nc.gpsimd.partition_broadcast(b2b_sb[:], b2row_sb[:])

            def phase_a(b):
                """QKV projections for batch b -> qk_sb (Q^T|K^T) and V1."""
                qk = qk_pool.tile([E, 2, S_], BF16, tag="qk")
                v1 = v1_pool.tile([128, KT, E + 1], BF16, tag="v1")
                nc.vector.memset(v1[:, :, E:E + 1], 1.0)
                for sc in range(NSC):
                    r0 = b * S_ + sc * 512
                    xT = xT_pool.tile([128, DJ, 512], BF16, tag="xT")
                    for dj in range(DJ):
                        nc.sync.dma_start_transpose(
                            xT[:, dj, :],
                            xb[r0:r0 + 512, dj * 128:(dj + 1) * 128])
                    # Q^T and K^T chunks
                    for idx, w_sb in ((0, wq_sb), (1, wk_sb)):
                        ps = ps_a.tile([E, 512], F32, tag="ps_a")
                        for dj in range(DJ):
                            nc.tensor.matmul(
                                ps[:], lhsT=w_sb[:, dj, :], rhs=xT[:, dj, :],
                                start=(dj == 0), stop=(dj == DJ - 1))
                        dst = qk[:, idx, sc * 512:(sc + 1) * 512]
                        if idx == 0:
                            nc.scalar.activation(dst, ps[:], AF.Identity,
                                                 bias=bq_sb[:, 0:1])
                        else:
                            nc.vector.tensor_copy(dst, ps[:])
                    # V in [s, e] layout
                    for sj in range(4):
                        psv = ps_a.tile([128, E], F32, tag="ps_a")
                        for dj in range(DJ):
                            nc.tensor.matmul(
                                psv[:],
                                lhsT=xT[:, dj, sj * 128:(sj + 1) * 128],
                                rhs=wv_sb[:, dj, :],
                                start=(dj == 0), stop=(dj == DJ - 1))
                        nc.vector.tensor_copy(v1[:, sc * 4 + sj, 0:E], psv[:])
                return qk, v1

            def phase_b(b, qk, v1):
                """Attention + wo projection for batch b -> y_bounce[b]."""
                for qi in range(N_CORES):
                    q0 = qi * QC
                    psc = ps_c.tile([E + 1, QC], F32, tag="ps_c")
                    for kt in range(KT):
                        pss = ps_s.tile([128, QC], F32, tag="ps_s")
                        nc.tensor.matmul(
                            pss[:],
                            lhsT=qk[:, 1, kt * 128:(kt + 1) * 128],
                            rhs=qk[:, 0, q0:q0 + QC],
                            start=True, stop=True)
                        es = exp_pool.tile([128, QC], BF16, tag="exp")
                        nc.scalar.activation(es[:], pss[:], AF.Exp, scale=0.125)
                        nc.tensor.matmul(
                            psc[:], lhsT=v1[:, kt, :], rhs=es[:],
                            start=(kt == 0), stop=(kt == KT - 1))
                    ctx_sb = ctx_pool.tile([E, QC], BF16, tag="ctx_sb")
                    nc.vector.tensor_copy(ctx_sb[:], psc[0:E, :])
                    recip = misc_pool.tile([1, QC], F32, tag="recip")
                    nc.vector.reciprocal(recip[:], psc[E:E + 1, :])
                    rb = misc_pool.tile([128, QC], F32, tag="rb")
                    nc.gpsimd.partition_broadcast(rb[:], recip[:])
                    for dj in range(DJ):
                        psy = ps_a.tile([128, QC], F32, tag="ps_a")
                        nc.tensor.matmul(
                            psy[:], lhsT=wo_sb[:, dj * 128:(dj + 1) * 128],
                            rhs=ctx_sb[:], start=True, stop=True)
                        y_sb = y_pool.tile([128, QC], F32, tag="y")
                        nc.vector.tensor_mul(y_sb[:], psy[:], rb[:])
                        nc.sync.dma_start(
                            y_bounce[b][qi, dj * 128:(dj + 1) * 128, :],
                            y_sb[:])

            def reduce_scatter(b):
                nc.gpsimd.collective_compute(
                    "ReduceScatter", mybir.AluOpType.add,
                    replica_groups=groups,
                    ins=[y_bounce[b][:, :, :].opt()],
                    outs=[rs_out[b][:, :].opt()])

            def mlp(b):
                """MLP on this core's row-slice of batch b -> out rows."""
                ysl = mlp_pool.tile([128, DJ, QC], F32, tag="ysl")
                nc.sync.dma_start(
                    ysl[:], rs_out[b].rearrange("(a p) q -> p a q", p=128))
                yslb = mlp_pool.tile([128, DJ, QC], BF16, tag="yslb")
                nc.vector.tensor_copy(yslb[:], ysl[:])
                h1 = mlp_pool.tile([128, DJ, QC], BF16, tag="h1")
                for dhj in range(DJ):
                    psh = ps_s.tile([128, QC], F32, tag="ps_s")
                    for dj in range(DJ):
                        nc.tensor.matmul(
                            psh[:],
                            lhsT=w1_sb[:, dj, dhj * 128:(dhj + 1) * 128],
                            rhs=yslb[:, dj, :],
                            start=(dj == 0), stop=(dj == DJ - 1))
                    nc.scalar.activation(h1[:, dhj, :], psh[:], AF.Relu,
                                         bias=b1e_sb[:, dhj:dhj + 1])
                for qj in range(QC // 128):
                    pso = ps_c.tile([128, D], F32, tag="ps_c")
                    for dhj in range(DJ):
                        nc.tensor.matmul(
                            pso[:],
                            lhsT=h1[:, dhj, qj * 128:(qj + 1) * 128],
                            rhs=w2_sb[:, dhj, :],
                            start=(dhj == 0), stop=(dhj == DJ - 1))
                    o_sb = y_pool.tile([128, D], F32, tag="osb")
                    nc.vector.tensor_add(o_sb[:], pso[:], b2b_sb[:])
                    nc.sync.dma_start(
                        out[b * QC + qj * 128: b * QC + (qj + 1) * 128, :],
                        o_sb[:])

            for b in range(B_):
                qk, v1 = phase_a(b)
                phase_b(b, qk, v1)
                reduce_scatter(b)
            for b in range(B_):
                mlp(b)

    nc.compile()
    return nc


def make_in_maps(inputs, B_=B, S_=S):
    """Host-side prep: cast/fold weights, build the 8 per-core input maps."""
    bf16 = ml_dtypes.bfloat16
    x = np.asarray(inputs["x"], np.float32).reshape(B_ * S_, D)
    wq, bq_ = np.asarray(inputs["wq"], np.float32), np.asarray(inputs["bq"], np.float32)
    wk = np.asarray(inputs["wk"], np.float32)
    wv, bv_ = np.asarray(inputs["wv"], np.float32), np.asarray(inputs["bv"], np.float32)
    wo, bo_ = np.asarray(inputs["wo"], np.float32), np.asarray(inputs["bo"], np.float32)
    w1_, b1_ = np.asarray(inputs["w1"], np.float32), np.asarray(inputs["b1"], np.float32)
    w2_, b2_ = np.asarray(inputs["w2"], np.float32), np.asarray(inputs["b2"], np.float32)

    # bv/bo fold: y_true = y_dev + c with c = sum_h bo_h + sum_h bv_h @ wo_h
    # (sum_k softmax = 1 makes the bv term exact); then b1_eff = b1 + c @ w1.
    c = bo_.sum(axis=0) + np.einsum("he,hed->d", bv_, wo)
    b1e_ = (b1_ + c @ w1_).astype(np.float32)

    xb = x.astype(bf16)
    in_maps = []
    for h in range(N_CORES):
        in_maps.append({
            "xb": xb,
            "wq": wq[h].astype(bf16),
            "wk": wk[h].astype(bf16),
            "wv": wv[h].astype(bf16),
            "bq": bq_[h].reshape(E, 1).astype(np.float32),
            "wo": wo[h].astype(bf16),
            "w1": w1_.astype(bf16),
            "b1e": b1e_.reshape(D, 1),
            "w2": w2_.astype(bf16),
            "b2": b2_.reshape(1, D).astype(np.float32),
        })
    return in_maps


def assemble_out(results, B_=B, S_=S):
    """Stitch the per-core row-slices back into [B, S, D]."""
    QC = S_ // N_CORES
    out = np.empty((B_, S_, D), np.float32)
    for i, r in enumerate(results):
        o = np.asarray(r["out"], np.float32)
        for b in range(B_):
            out[b, i * QC:(i + 1) * QC, :] = o[b * QC:(b + 1) * QC, :]
    return out


_CACHED = {}


def kernel(**inputs) -> np.ndarray:
    from concourse.bass_utils import run_bass_kernel_spmd
    if "nc" not in _CACHED:
        _CACHED["nc"] = build()
    nc = _CACHED["nc"]
    in_maps = make_in_maps(inputs)
    res = run_bass_kernel_spmd(nc, in_maps, core_ids=list(range(N_CORES)))
    return assemble_out(res.results)


if __name__ == "__main__":
    import jax
    rng = np.random.default_rng(0)
    print("building...")
    nc = build()
    print("built ok")


# revision 7
# speedup vs baseline: 1.7857x; 1.7857x over previous
"""Trainium2 Bass kernel for nn_AttentionLayer (B=2, S=4096, D=512, H=8, E=64).

Strategy: head-parallel over 8 NeuronCores. Each core computes one head's
Q/K/V projections + attention + output projection for the full sequence,
then a ReduceScatter (per batch) sums the per-head partial outputs and
shards the summed rows across cores; each core runs the (replicated) MLP
on its row-slice and writes its slice of the final output.

Layout notes (everything is "feature on partitions" until the MLP output):
  x^T tiles  [d(128), s]   via DMA-transpose of host-pre-cast bf16 x
  Q^T / K^T  [e(64),  s]   packed in one [128, S] tile (rows 0:64 = Q^T,
                           64:128 = K^T)
  V1 tiles   [k(128), 65]  V with a ones-column appended so the attention
                           AV matmul also produces the softmax denominator
  scores^T   [k(128), q]   exp() applied on PSUM eviction (no max-subtract:
                           inputs are bounded so exp cannot overflow)
  y^T        [d(128), q]   normalized after the wo projection (divide by
                           denominator commutes with the linear projection)

Exact algebra folds (no approximation): bk drops out of softmax entirely;
bv and bo are folded into an effective MLP b1 on the host (b1_eff), since
sum_k softmax = 1 and the MLP consumes y linearly before relu.
"""

import numpy as np
import ml_dtypes

import concourse.bass as bass
import concourse.bacc as bacc
import concourse.mybir as mybir
import concourse.tile as tile

F32 = mybir.dt.float32
BF16 = mybir.dt.bfloat16
AF = mybir.ActivationFunctionType

B, S, D, H, E = 2, 4096, 512, 8, 64
N_CORES = 8


def build(B_=B, S_=S, debug=False):
    """Build the per-core SPMD graph. All 8 cores run the identical graph;
    per-head behavior comes from the per-core weight shards."""
    QC = S_ // N_CORES          # per-core slice rows per batch == q-chunk
    KT = S_ // 128              # number of 128-row key tiles
    DJ = D // 128               # 4 chunks of the hidden dim
    NSC = S_ // 512             # phase-A s-chunks of 512 rows
    assert QC % 128 == 0 and QC <= 512 and S_ % 512 == 0

    nc = bacc.Bacc(None, target_bir_lowering=False, debug=debug,
                   num_devices=N_CORES)

    xb = nc.declare_dram_parameter("xb", [B_ * S_, D], BF16, False)
    wq = nc.declare_dram_parameter("wq", [D, E], BF16, False)
    wk = nc.declare_dram_parameter("wk", [D, E], BF16, False)
    wv = nc.declare_dram_parameter("wv", [D, E], BF16, False)
    bq = nc.declare_dram_parameter("bq", [E, 1], F32, False)
    wo = nc.declare_dram_parameter("wo", [E, D], BF16, False)
    w1 = nc.declare_dram_parameter("w1", [D, D], BF16, False)
    b1e = nc.declare_dram_parameter("b1e", [D, 1], F32, False)
    w2 = nc.declare_dram_parameter("w2", [D, D], BF16, False)
    b2 = nc.declare_dram_parameter("b2", [1, D], F32, False)
    out = nc.declare_dram_parameter("out", [B_ * QC, D], F32, True)

    # Internal DRAM bounce buffers for the collectives.
    y_bounce = [nc.dram_tensor(f"y_bounce{b}", [N_CORES, D, QC], F32)
                for b in range(B_)]
    rs_out = [nc.dram_tensor(f"rs_out{b}", [D, QC], F32)
              for b in range(B_)]
    groups = [list(range(N_CORES))]

    with tile.TileContext(nc) as tc:
        from contextlib import ExitStack
        with ExitStack() as ctx:
            consts = ctx.enter_context(tc.tile_pool(name="consts", bufs=1))
            xT_pool = ctx.enter_context(tc.tile_pool(name="xT", bufs=2))
            qk_pool = ctx.enter_context(tc.tile_pool(name="qk", bufs=2))
            v1_pool = ctx.enter_context(tc.tile_pool(name="v1", bufs=2))
            exp_pool = ctx.enter_context(tc.tile_pool(name="exp", bufs=3))
            ctx_pool = ctx.enter_context(tc.tile_pool(name="ctxp", bufs=2))
            y_pool = ctx.enter_context(tc.tile_pool(name="y", bufs=4))
            mlp_pool = ctx.enter_context(tc.tile_pool(name="mlp", bufs=2))
            misc_pool = ctx.enter_context(tc.tile_pool(name="misc", bufs=2))
            ps_a = ctx.enter_context(
                tc.tile_pool(name="ps_a", bufs=2, space="PSUM"))
            ps_s = ctx.enter_context(
                tc.tile_pool(name="ps_s", bufs=2, space="PSUM"))
            ps_c = ctx.enter_context(
                tc.tile_pool(name="ps_c", bufs=2, space="PSUM"))

            # ---- constants / weights into SBUF ----
            wq_sb = consts.tile([128, DJ, E], BF16, tag="wq")
            wk_sb = consts.tile([128, DJ, E], BF16, tag="wk")
            wv_sb = consts.tile([128, DJ, E], BF16, tag="wv")
            nc.sync.dma_start(wq_sb[:], wq.rearrange("(a p) e -> p a e", p=128))
            nc.sync.dma_start(wk_sb[:], wk.rearrange("(a p) e -> p a e", p=128))
            nc.sync.dma_start(wv_sb[:], wv.rearrange("(a p) e -> p a e", p=128))
            bq_sb = consts.tile([E, 1], F32, tag="bq")
            nc.sync.dma_start(bq_sb[:], bq[:, :])
            wo_sb = consts.tile([E, D], BF16, tag="wo")
            nc.sync.dma_start(wo_sb[:], wo[:, :])
            w1_sb = consts.tile([128, DJ, D], BF16, tag="w1")
            nc.sync.dma_start(w1_sb[:], w1.rearrange("(a p) m -> p a m", p=128))
            w2_sb = consts.tile([128, DJ, D], BF16, tag="w2")
            nc.sync.dma_start(w2_sb[:], w2.rearrange("(a p) m -> p a m", p=128))
            b1e_sb = consts.tile([128, DJ], F32, tag="b1e")
            nc.sync.dma_start(b1e_sb[:], b1e.rearrange("(a p) o -> p (a o)", p=128))
            b2row_sb = consts.tile([1, D], F32, tag="b2row")
            nc.sync.dma_start(b2row_sb[:], b2[:, :])
            b2b_sb = consts.tile([128, D], F32, tag="b2b")
            nc.gpsimd.partition_broadcast(b2b_sb[:], b2row_sb[:])

            def phase_a(b):
                """QKV projections for batch b -> qk_sb (Q^T|K^T) and V1."""
                qk = qk_pool.tile([E, 2, S_], BF16, tag="qk")
                v1 = v1_pool.tile([128, KT, E + 1], BF16, tag="v1")
                nc.vector.memset(v1[:, :, E:E + 1], 1.0)
                for sc in range(NSC):
                    r0 = b * S_ + sc * 512
                    xT = xT_pool.tile([128, DJ, 512], BF16, tag="xT")
                    for dj in range(DJ):
                        nc.sync.dma_start_transpose(
                            xT[:, dj, :],
                            xb[r0:r0 + 512, dj * 128:(dj + 1) * 128])
                    # Q^T and K^T chunks
                    for idx, w_sb in ((0, wq_sb), (1, wk_sb)):
                        ps = ps_a.tile([E, 512], F32, tag="ps_a")
                        for dj in range(DJ):
                            nc.tensor.matmul(
                                ps[:], lhsT=w_sb[:, dj, :], rhs=xT[:, dj, :],
                                start=(dj == 0), stop=(dj == DJ - 1))
                        dst = qk[:, idx, sc * 512:(sc + 1) * 512]
                        if idx == 0:
                            nc.vector.tensor_scalar_add(dst, ps[:],
                                                        bq_sb[:, 0:1])
                        else:
                            nc.vector.tensor_copy(dst, ps[:])
                    # V in [s, e] layout
                    for sj in range(4):
                        psv = ps_a.tile([128, E], F32, tag="ps_a")
                        for dj in range(DJ):
                            nc.tensor.matmul(
                                psv[:],
                                lhsT=xT[:, dj, sj * 128:(sj + 1) * 128],
                                rhs=wv_sb[:, dj, :],
                                start=(dj == 0), stop=(dj == DJ - 1))
                        nc.vector.tensor_copy(v1[:, sc * 4 + sj, 0:E], psv[:])
                return qk, v1

            def phase_b(b, qk, v1):
                """Attention + wo projection for batch b -> y_bounce[b]."""
                for qi in range(N_CORES):
                    q0 = qi * QC
                    psc = ps_c.tile([E + 1, QC], F32, tag="ps_c")
                    for kt2 in range(KT // 2):
                        # pair two key tiles per exp instruction: one 2-bank
                        # PSUM tile, one [128, 2*QC] activation
                        pss = ps_s.tile([128, 2, QC], F32, tag="ps_s")
                        for j in range(2):
                            kt = kt2 * 2 + j
                            nc.tensor.matmul(
                                pss[:, j, :],
                                lhsT=qk[:, 1, kt * 128:(kt + 1) * 128],
                                rhs=qk[:, 0, q0:q0 + QC],
                                start=True, stop=True)
                        es = exp_pool.tile([128, 2, QC], BF16, tag="exp")
                        nc.scalar.activation(es[:], pss[:], AF.Exp, scale=0.125)
                        for j in range(2):
                            kt = kt2 * 2 + j
                            nc.tensor.matmul(
                                psc[:], lhsT=v1[:, kt, :], rhs=es[:, j, :],
                                start=(kt == 0), stop=(kt == KT - 1))
                    ctx_sb = ctx_pool.tile([E, QC], BF16, tag="ctx_sb")
                    nc.vector.tensor_copy(ctx_sb[:], psc[0:E, :])
                    recip = misc_pool.tile([1, QC], F32, tag="recip")
                    nc.vector.reciprocal(recip[:], psc[E:E + 1, :])
                    rb = misc_pool.tile([128, QC], F32, tag="rb")
                    nc.gpsimd.partition_broadcast(rb[:], recip[:])
                    y_sb = y_pool.tile([128, DJ, QC], F32, tag="y")
                    for dj in range(DJ):
                        psy = ps_a.tile([128, QC], F32, tag="ps_a")
                        nc.tensor.matmul(
                            psy[:], lhsT=wo_sb[:, dj * 128:(dj + 1) * 128],
                            rhs=ctx_sb[:], start=True, stop=True)
                        nc.vector.tensor_mul(y_sb[:, dj, :], psy[:], rb[:])
                    nc.sync.dma_start(
                        y_bounce[b][qi].rearrange("(a p) q -> p a q", p=128),
                        y_sb[:])

            def reduce_scatter(b):
                nc.gpsimd.collective_compute(
                    "ReduceScatter", mybir.AluOpType.add,
                    replica_groups=groups,
                    ins=[y_bounce[b][:, :, :].opt()],
                    outs=[rs_out[b][:, :].opt()])

            def mlp(b):
                """MLP on this core's row-slice of batch b -> out rows."""
                ysl = mlp_pool.tile([128, DJ, QC], F32, tag="ysl")
                nc.sync.dma_start(
                    ysl[:], rs_out[b].rearrange("(a p) q -> p a q", p=128))
                yslb = mlp_pool.tile([128, DJ, QC], BF16, tag="yslb")
                nc.vector.tensor_copy(yslb[:], ysl[:])
                h1 = mlp_pool.tile([128, DJ, QC], BF16, tag="h1")
                for dhj in range(DJ):
                    psh = ps_s.tile([128, QC], F32, tag="ps_s")
                    for dj in range(DJ):
                        nc.tensor.matmul(
                            psh[:],
                            lhsT=w1_sb[:, dj, dhj * 128:(dhj + 1) * 128],
                            rhs=yslb[:, dj, :],
                            start=(dj == 0), stop=(dj == DJ - 1))
                    nc.vector.tensor_scalar(
                        h1[:, dhj, :], psh[:], b1e_sb[:, dhj:dhj + 1], 0.0,
                        op0=mybir.AluOpType.add, op1=mybir.AluOpType.max)
                for qj in range(QC // 128):
                    pso = ps_c.tile([128, D], F32, tag="ps_c")
                    for dhj in range(DJ):
                        nc.tensor.matmul(
                            pso[:],
                            lhsT=h1[:, dhj, qj * 128:(qj + 1) * 128],
                            rhs=w2_sb[:, dhj, :],
                            start=(dhj == 0), stop=(dhj == DJ - 1))
                    o_sb = y_pool.tile([128, D], F32, tag="osb")
                    nc.vector.tensor_add(o_sb[:], pso[:], b2b_sb[:])
                    nc.sync.dma_start(
                        out[b * QC + qj * 128: b * QC + (qj + 1) * 128, :],
                        o_sb[:])

            for b in range(B_):
                qk, v1 = phase_a(b)
                phase_b(b, qk, v1)
                reduce_scatter(b)
            for b in range(B_):
                mlp(b)

    nc.compile()
    return nc


def make_in_maps(inputs, B_=B, S_=S):
    """Host-side prep: cast/fold weights, build the 8 per-core input maps."""
    bf16 = ml_dtypes.bfloat16
    x = np.asarray(inputs["x"], np.float32).reshape(B_ * S_, D)
    wq, bq_ = np.asarray(inputs["wq"], np.float32), np.asarray(inputs["bq"], np.float32)
    wk = np.asarray(inputs["wk"], np.float32)
    wv, bv_ = np.asarray(inputs["wv"], np.float32), np.asarray(inputs["bv"], np.float32)
    wo, bo_ = np.asarray(inputs["wo"], np.float32), np.asarray(inputs["bo"], np.float32)
    w1_, b1_ = np.asarray(inputs["w1"], np.float32), np.asarray(inputs["b1"], np.float32)
    w2_, b2_ = np.asarray(inputs["w2"], np.float32), np.asarray(inputs["b2"], np.float32)

    # bv/bo fold: y_true = y_dev + c with c = sum_h bo_h + sum_h bv_h @ wo_h
    # (sum_k softmax = 1 makes the bv term exact); then b1_eff = b1 + c @ w1.
    c = bo_.sum(axis=0) + np.einsum("he,hed->d", bv_, wo)
    b1e_ = (b1_ + c @ w1_).astype(np.float32)

    xb = x.astype(bf16)
    in_maps = []
    for h in range(N_CORES):
        in_maps.append({
            "xb": xb,
            "wq": wq[h].astype(bf16),
            "wk": wk[h].astype(bf16),
            "wv": wv[h].astype(bf16),
            "bq": bq_[h].reshape(E, 1).astype(np.float32),
            "wo": wo[h].astype(bf16),
            "w1": w1_.astype(bf16),
            "b1e": b1e_.reshape(D, 1),
            "w2": w2_.astype(bf16),
            "b2": b2_.reshape(1, D).astype(np.float32),
        })
    return in_maps


def assemble_out(results, B_=B, S_=S):
    """Stitch the per-core row-slices back into [B, S, D]."""
    QC = S_ // N_CORES
    out = np.empty((B_, S_, D), np.float32)
    for i, r in enumerate(results):
        o = np.asarray(r["out"], np.float32)
        for b in range(B_):
            out[b, i * QC:(i + 1) * QC, :] = o[b * QC:(b + 1) * QC, :]
    return out


_CACHED = {}


def kernel(**inputs) -> np.ndarray:
    from concourse.bass_utils import run_bass_kernel_spmd
    if "nc" not in _CACHED:
        _CACHED["nc"] = build()
    nc = _CACHED["nc"]
    in_maps = make_in_maps(inputs)
    res = run_bass_kernel_spmd(nc, in_maps, core_ids=list(range(N_CORES)))
    return assemble_out(res.results)


if __name__ == "__main__":
    import jax
    rng = np.random.default_rng(0)
    print("building...")
    nc = build()
    print("built ok")
